# revision 1
# baseline (speedup 1.0000x reference)
"""Trainium2 Bass kernel: binarized (XNOR/ReActNet-style) ResNet BasicBlock.

Computes, for x:[64,64,56,56] f32 and small per-channel parameters:

    out = PReLU_a(BN(conv3x3(sign(x + b0), scale * sign(w))) + x + b1) + b2

Distribution: data-parallel over the batch dim, 8 images per NeuronCore on
8 cores.  Per core, images (i, i+4) share the SBUF partition dim: channels
of the first image on partitions 0-63, channels of the second on 64-127.

Math folding (host side, all tiny tensors):
  - binarized weights sign(w) are pre-scaled by A_m = mean|w|_m * gamma_m /
    sqrt(var_m + eps)  (the BN multiplier), so PSUM holds BN-scaled conv.
    Products are +-A_m exactly, accumulated in fp32 PSUM -> only error is
    bf16 rounding of A_m itself (~2^-9 relative).
  - residual +x is accumulated into PSUM by an identity matmul streaming
    bf16 x planes; the PE quadrant crossing aligns it with the conv halves.
    The bf16 staging copies run on the otherwise-idle GpSimd (Pool) engine.
  - C2_m = beta - mean*inv + bias1 is applied in the PSUM-drain DVE op.
  - PReLU runs on the ScalarE as parametric_relu with per-channel alpha;
    bias2 (zero in practice) falls back to one extra DVE op when nonzero.

Engine budget per core/pass: PE streams 9 bf16 taps x 464 + 1 bf16 x 448
over 4 quadrant streams (~33us), DMA moves 12.8MB (~39us, the roofline),
ACT does sign + prelu (~27us), DVE drains PSUM +C2 (~17us), Pool casts
x to bf16 (~11us).  All DMAs ride the SP HWDGE ring: 8 input loads
up-front, per-pair output stores behind them; the ring drains
back-to-back so the DMA engines never idle.

On-chip layout: activations live in zero-padded 58x58 bf16 planes so each
3x3 tap is one contiguous 464-element matmul rhs slice; x and y live in
unpadded planes so HBM DMAs are 64 descriptors x 12.5KB contiguous.
Conv runs as 9 small matmuls per 8-row slice on 2x2 PE quadrants
(tile_position from partition bases); even/odd slices use complementary
quadrant pairs so four matmul streams run concurrently.
"""

import sys

if "/opt/trn_rl_repo" not in sys.path:
    sys.path.insert(0, "/opt/trn_rl_repo")

import numpy as np

import concourse.bass as bass
import concourse.bacc as bacc
import concourse.mybir as mybir
from concourse.tile import TileContext
from concourse.bass_utils import run_bass_kernel_spmd

AF = mybir.ActivationFunctionType
ALU = mybir.AluOpType
DT = mybir.dt

B, C, H, W = 64, 64, 56, 56
NCORES = 8
BPC = B // NCORES          # images per core
NPAIR = BPC // 2           # image pairs per core
HP, WP = H + 2, W + 2      # zero-padded plane 58x58
IMG = HP * WP              # 3364 elements per padded plane
PLN = H * W                # 3136 elements per unpadded plane
RB = 8                     # output rows per slice
NSL = H // RB              # 7 slices per image
NT = RB * WP               # 464: matmul free size (contiguous in padded space)
NI = RB * W                # 448: interior (valid) elements per slice
BN_EPS = 1e-5

_NC_CACHE = {}


def _build(fast_prelu: bool, reps: int = 1, probe: str = ''):
    nc = bacc.Bacc("TRN2", target_bir_lowering=False, debug=False)
    # pair-major layout: images (i, i+NPAIR) adjacent so one 128-partition
    # DMA covers a pair (host interleaves/deinterleaves)
    x_ext = nc.declare_dram_parameter("x", [NPAIR, 2, C, H, W], DT.float32,
                                      isOutput=False)
    w_ext = nc.declare_dram_parameter("wts", [128, 9 * 64], DT.bfloat16, isOutput=False)
    i_ext = nc.declare_dram_parameter("wid", [128, 64], DT.bfloat16, isOutput=False)
    c_ext = nc.declare_dram_parameter("cst", [128, 8], DT.float32, isOutput=False)
    o_ext = nc.declare_dram_parameter("out", [NPAIR, 2, C, H, W], DT.float32,
                                      isOutput=True)

    xg = x_ext.ap().rearrange("p b c h w -> p (b c) (h w)")
    og = o_ext.ap().rearrange("p b c h w -> p (b c) (h w)")

    with TileContext(nc) as tc:
        with tc.tile_pool(name="persist", bufs=1) as perst, \
             tc.tile_pool(name="work", bufs=4) as work, \
             tc.tile_pool(name="psum", bufs=6, space="PSUM") as ppool:

            wts = perst.tile([128, 9 * 64], DT.bfloat16)
            nc.sync.dma_start(out=wts, in_=w_ext.ap())
            wid = perst.tile([128, 64], DT.bfloat16)
            nc.sync.dma_start(out=wid, in_=i_ext.ap())
            cst = perst.tile([128, 8], DT.float32)
            nc.sync.dma_start(out=cst, in_=c_ext.ap())
            c_ap = cst[:, 0:1]    # beta - mean*inv + bias1
            a_ap = cst[:, 1:2]    # PReLU alpha
            b2_ap = cst[:, 2:3]   # bias2 (nonzero only on the slow path)
            b0_ap = cst[:, 3:4]   # bias0

            xf = perst.tile([128, NPAIR * PLN], DT.float32)    # unpadded planes
            xb = perst.tile([128, NPAIR * PLN], DT.bfloat16)   # bf16 residual
            yb = perst.tile([128, NPAIR * PLN], DT.float32)    # unpadded output
            act = perst.tile([128, NPAIR * IMG], DT.bfloat16)  # padded sign planes

            if probe == 'pe':
                # mark xb written so the PE-only probe's identity matmuls
                # have an allocated (garbage) source
                nc.vector.memset(xb[:, 0:1], 0.0)
            if probe in ('dma', 'dma2'):
                # mark yb written: the DMA-only probe stores garbage yb so
                # loads (xf) and stores (yb) stay WAR-independent, matching
                # the real kernel's conveyor structure
                nc.vector.memset(yb[:, 0:1], 0.0)

            # zero the padding ring of every act plane
            for pr in range(NPAIR):
                v = act[:, pr * IMG:(pr + 1) * IMG].rearrange(
                    "p (h w) -> p h w", w=WP)
                nc.vector.memset(v[:, 0:1, :], 0.0)
                nc.vector.memset(v[:, HP - 1:HP, :], 0.0)
                nc.vector.memset(v[:, 1:HP - 1, 0:1], 0.0)
                nc.vector.memset(v[:, 1:HP - 1, WP - 1:WP], 0.0)

            def emit_load(pr):
                ub = pr * PLN
                nc.sync.dma_start(out=xf[:, ub:ub + PLN], in_=xg[pr])

            def emit_xb_cast(pr):
                # bf16 residual staging on the GpSimd engine (idle otherwise),
                # two chunks per pair for finer pipeline granularity
                ub = pr * PLN
                hh = PLN // 2
                nc.gpsimd.tensor_copy(out=xb[:, ub:ub + hh],
                                      in_=xf[:, ub:ub + hh])
                nc.gpsimd.tensor_copy(out=xb[:, ub + hh:ub + PLN],
                                      in_=xf[:, ub + hh:ub + PLN])

            def emit_sign(pr, half):
                # sign of a half-image (28 rows) in one ACT instruction
                ub = pr * PLN
                ab = pr * IMG
                h0 = half * (H // 2)
                nr = H // 2
                src = xf[:, ub + h0 * W:ub + (h0 + nr) * W].rearrange(
                    "p (r c) -> p r c", c=W)
                adst = act[:, ab:ab + IMG].rearrange(
                    "p (h w) -> p h w", w=WP)[:, 1 + h0:1 + h0 + nr, 1:1 + W]
                nc.scalar.activation(out=adst, in_=src, func=AF.Sign,
                                     bias=b0_ap, scale=1.0)

            def mm_args(s, t):
                pr, rc = divmod(s, NSL)
                h0 = rc * RB
                if t == 9:      # identity (residual) step: bf16 from xb
                    u0 = pr * PLN + h0 * W
                    la = wid[0:64]
                    lb = wid[64:128]
                    ra = xb[0:64, u0:u0 + NI]
                    rb = xb[64:128, u0:u0 + NI]
                    return la, lb, ra, rb, None, None, s % 2
                dh, dw = t // 3 - 1, t % 3 - 1
                off = pr * IMG + (h0 + dh + 1) * WP + dw
                s0 = 1 if (rc == 0 and dh == -1 and dw == -1) else 0
                s1 = NT - 1 if (rc == NSL - 1 and dh == 1 and dw == 1) else NT
                la = wts[0:64, t * 64:(t + 1) * 64]
                lb = wts[64:128, t * 64:(t + 1) * 64]
                ra = act[0:64, off + s0:off + s1]
                rb = act[64:128, off + s0:off + s1]
                return la, lb, ra, rb, s0, s1, s % 2

            def emit_mms(slice_group):
                # interleave matmuls of an even+odd slice pair so all four
                # PE quadrants stream concurrently (starts are pc-monotone;
                # disjoint tile_positions overlap)
                pss = {}
                for s in slice_group:
                    pss[s] = ppool.tile([128, NT], DT.float32, tag="ps",
                                        name=f"ps{s}")
                # center tap first: never range-trimmed, so start=True clears
                # the whole bank before the trimmed corner taps accumulate
                for t in (4, 0, 1, 2, 3, 5, 6, 7, 8, 9):
                    for s in slice_group:
                        la, lb, ra, rb, s0, s1, odd = mm_args(s, t)
                        ps = pss[s]
                        if t == 9:
                            # write the 448-elem interior (rows of 56 within
                            # the 58-wide padded window)
                            pv = ps.rearrange("p (r c) -> p r c", c=WP)[:, :, 1:1 + W]
                            pa = pv[64:128] if odd else pv[0:64]
                            pb = pv[0:64] if odd else pv[64:128]
                        else:
                            pa = ps[64:128, s0:s1] if odd else ps[0:64, s0:s1]
                            pb = ps[0:64, s0:s1] if odd else ps[64:128, s0:s1]
                        nc.tensor.matmul(pa, la, ra, start=(t == 4),
                                         stop=(t == 9), skip_group_check=True)
                        nc.tensor.matmul(pb, lb, rb, start=(t == 4),
                                         stop=(t == 9), skip_group_check=True)
                return pss

            def emit_epilogue(s, ps):
                pr, rc = divmod(s, NSL)
                h0 = rc * RB
                u0 = pr * PLN + h0 * W
                tt = work.tile([128, NI], DT.float32, tag="tt")
                ps_i = ps.rearrange("p (r c) -> p r c", c=WP)[:, :, 1:1 + W]
                tt_v = tt.rearrange("p (r c) -> p r c", c=W)
                # u = psum + C2   (PSUM drain; conv+BN+residual already in ps)
                nc.vector.tensor_scalar_add(tt_v, ps_i, c_ap)
                yv = yb[:, u0:u0 + NI]
                # y = prelu_a(u)  (exact per-channel PReLU on ScalarE)
                nc.scalar.activation(out=yv, in_=tt, func=AF.Prelu,
                                     bias=0.0, scale=1.0, alpha=a_ap)
                if not fast_prelu:
                    # + bias2 (only when nonzero)
                    nc.vector.tensor_scalar_add(yv, yv, b2_ap)

            def emit_store(pr, eng=None):
                # output store, split by slice parity: odd-parity slices have
                # swapped halves (image B on partitions 0-63) from the crossed
                # PE quadrants.  Normal-parity chunks go out as one merged
                # 128-partition DMA; swapped-parity chunks as two crossed
                # 64-partition DMAs.
                eng = eng if eng is not None else nc.sync
                ub = pr * PLN
                ov = og[pr].rearrange("q (k n) -> q k n", n=NI)
                yv = yb[:, ub:ub + PLN].rearrange("q (k n) -> q k n", n=NI)
                pn = pr % 2          # rc parity whose layout is normal [A|B]
                psw = 1 - pn
                eng.dma_start(out=ov[:, pn:NSL:2], in_=yv[:, pn:NSL:2])
                eng.dma_start(out=ov[0:64, psw:NSL:2], in_=yv[64:128, psw:NSL:2])
                eng.dma_start(out=ov[64:128, psw:NSL:2], in_=yv[0:64, psw:NSL:2])

            def emit_compute_all():
                if probe == 'pe':
                    for s0 in range(0, NPAIR * NSL - 1, 2):
                        emit_mms([s0, s0 + 1])
                    return
                if probe == 'dma':
                    for pr in range(NPAIR):
                        emit_store(pr)
                    return
                if probe == 'dma2':
                    # conveyor with stores on the second HWDGE ring (ACT)
                    for pr in range(NPAIR):
                        emit_store(pr, eng=nc.scalar)
                    return
                nsl_tot = NPAIR * NSL
                s = 0
                while s < nsl_tot:
                    group = [s] if s + 1 >= nsl_tot else [s, s + 1]
                    if probe == 'nope':
                        pss = {g: ppool.tile([128, NT], DT.float32, tag="ps",
                                             name=f"ps{g}") for g in group}
                        for g in group:
                            nc.vector.memset(pss[g][:, 0:1], 0.0)
                    else:
                        pss = emit_mms(group)
                    for g in group:
                        emit_epilogue(g, pss[g])
                    # prefetch next pair's signs through the ACT FIFO
                    # (queued behind this group's prelus so they never
                    # head-of-line block them on a pending input load):
                    # at slices 0 and 2 of pair pr, emit the two half-image
                    # signs of pair pr+1
                    for g in group:
                        pr_g, rc_g = divmod(g, NSL)
                        if rc_g in (0, 2) and pr_g + 1 < NPAIR:
                            emit_sign(pr_g + 1, rc_g // 2)
                    for g in group:
                        if (g + 1) % NSL == 0:
                            emit_store(g // NSL)
                    s += len(group)

            for _ in range(reps):
                if probe != 'pe':
                    # loads all up-front on the SP ring; pair-0 signs up-front
                    # (later pairs' signs are interleaved into the slice loop)
                    for pr in range(NPAIR):
                        emit_load(pr)
                    if probe not in ('dma', 'dma2'):
                        for pr in range(NPAIR):
                            emit_xb_cast(pr)
                        emit_sign(0, 0)
                        emit_sign(0, 1)
                emit_compute_all()

    nc.compile()
    return nc


def _get_nc(fast_prelu: bool, reps: int = 1, probe: str = ''):
    key = (fast_prelu, reps, probe)
    if key not in _NC_CACHE:
        _NC_CACHE[key] = _build(fast_prelu, reps, probe)
    return _NC_CACHE[key]


def _prepare(x, bias0, w, gamma, beta, run_mean, run_var, bias1, alpha, bias2):
    bf16 = DT.np(DT.bfloat16)
    x = np.ascontiguousarray(np.asarray(x, np.float32))
    w = np.asarray(w, np.float32)
    sw = np.sign(w)                                   # [P, C, 3, 3]
    scale = np.abs(w).mean(axis=(1, 2, 3))            # [P]
    inv = np.asarray(gamma, np.float32) / np.sqrt(
        np.asarray(run_var, np.float32) + np.float32(BN_EPS))
    A = (scale * inv).astype(np.float32)
    b1 = np.asarray(bias1, np.float32).reshape(-1)
    b2 = np.asarray(bias2, np.float32).reshape(-1)
    al = np.asarray(alpha, np.float32).reshape(-1)
    b0 = np.asarray(bias0, np.float32).reshape(-1)
    Cc = (np.asarray(beta, np.float32) -
          np.asarray(run_mean, np.float32) * inv + b1).astype(np.float32)

    wt = np.zeros((128, 9 * 64), np.float32)
    for t in range(9):
        blk = (sw[:, :, t // 3, t % 3] * A[:, None]).T      # [C, P]
        wt[0:64, t * 64:(t + 1) * 64] = blk
        wt[64:128, t * 64:(t + 1) * 64] = blk
    wt_bf = np.ascontiguousarray(wt.astype(bf16))
    wid = np.zeros((128, 64), np.float32)
    wid[0:64] = np.eye(64, dtype=np.float32)
    wid[64:128] = np.eye(64, dtype=np.float32)
    wid = np.ascontiguousarray(wid.astype(bf16))

    cst = np.zeros((128, 8), np.float32)
    for half in range(2):
        sl = slice(half * 64, half * 64 + 64)
        cst[sl, 0] = Cc
        cst[sl, 1] = al
        cst[sl, 2] = b2
        cst[sl, 3] = b0

    fast_prelu = bool(np.all(b2 == 0.0))
    in_maps = []
    for c in range(NCORES):
        xc = x[c * BPC:(c + 1) * BPC]
        # pair-major: [NPAIR, 2, C, H, W] with images (p, p+NPAIR) adjacent
        xpm = np.ascontiguousarray(
            np.stack([xc[0:NPAIR], xc[NPAIR:BPC]], axis=1))
        in_maps.append({"x": xpm, "wts": wt_bf, "wid": wid, "cst": cst})
    return in_maps, fast_prelu


_RUNNER_CACHE = {}


def _make_runner(nc, n_cores=NCORES):
    """Build a reusable jitted executor for `nc` (one XLA trace, NEFF cached)."""
    import jax
    from jax.sharding import Mesh, PartitionSpec, NamedSharding
    from jax.experimental.shard_map import shard_map
    from concourse import bass2jax

    bass2jax.install_neuronx_cc_hook()
    partition_name = nc.partition_id_tensor.name if nc.partition_id_tensor else None
    in_names, out_names, out_avals, zero_outs = [], [], [], []
    for alloc in nc.m.functions[0].allocations:
        if not isinstance(alloc, mybir.MemoryLocationSet):
            continue
        name = alloc.memorylocations[0].name
        if alloc.kind == "ExternalInput":
            if name != partition_name:
                in_names.append(name)
        elif alloc.kind == "ExternalOutput":
            out_names.append(name)
            shape = tuple(alloc.tensor_shape)
            dtype = mybir.dt.np(alloc.dtype)
            out_avals.append(jax.core.ShapedArray(shape, dtype))
            zero_outs.append(np.zeros(shape, dtype))
    n_params = len(in_names)
    all_in = list(in_names) + out_names + ([partition_name] if partition_name else [])

    def _body(*args):
        operands = list(args)
        if partition_name is not None:
            operands.append(bass2jax.partition_id_tensor())
        outs = bass2jax._bass_exec_p.bind(
            *operands,
            out_avals=tuple(out_avals),
            in_names=tuple(all_in),
            out_names=tuple(out_names),
            lowering_input_output_aliases=(),
            sim_require_finite=True,
            sim_require_nnan=True,
            nc=nc,
        )
        return tuple(outs)

    devices = jax.devices()[:n_cores]
    mesh = Mesh(np.asarray(devices), ("core",))
    nin = n_params + len(out_names)
    f = jax.jit(shard_map(
        _body, mesh=mesh,
        in_specs=(PartitionSpec("core"),) * nin,
        out_specs=(PartitionSpec("core"),) * len(out_names),
        check_rep=False))
    sh = NamedSharding(mesh, PartitionSpec("core"))
    concat_zeros = [
        jax.device_put(np.zeros((n_cores * z.shape[0], *z.shape[1:]), z.dtype), sh)
        for z in zero_outs
    ]

    def run(in_maps):
        concat_in = [
            np.concatenate([np.asarray(in_maps[c][nm]) for c in range(n_cores)],
                           axis=0)
            for nm in in_names
        ]
        args = [jax.device_put(a, sh) for a in concat_in] + concat_zeros
        outs = f(*args)
        jax.block_until_ready(outs)
        oi = out_names.index("out")
        full = np.asarray(outs[oi])
        return full.reshape(n_cores, *out_avals[oi].shape)

    run.jit_fn = f
    run.sharding = sh
    run.in_names = in_names
    run.out_names = out_names
    run.zero_args = concat_zeros
    return run


def _get_runner(fast_prelu: bool, reps: int = 1, probe: str = ''):
    key = (fast_prelu, reps, probe)
    if key not in _RUNNER_CACHE:
        _RUNNER_CACHE[key] = _make_runner(_get_nc(fast_prelu, reps, probe))
    return _RUNNER_CACHE[key]


def _run(inputs: dict, trace: bool = False, reps: int = 1, **spmd_kwargs):
    """Legacy path through run_bass_kernel_spmd (used for debugging)."""
    in_maps, fast_prelu = _prepare(**inputs)
    nc = _get_nc(fast_prelu, reps)
    res = run_bass_kernel_spmd(nc, in_maps, list(range(NCORES)),
                               trace=trace, **spmd_kwargs)
    per_core = np.stack([res.results[c]["out"] for c in range(NCORES)], axis=0)
    out = np.transpose(per_core, (0, 2, 1, 3, 4, 5)).reshape(B, C, H, W)
    return out, res


def kernel(**inputs) -> np.ndarray:
    in_maps, fast_prelu = _prepare(**inputs)
    runner = _get_runner(fast_prelu)
    per_core = runner(in_maps)      # [NCORES, NPAIR, 2, C, H, W]
    # undo pair-major: image b of core c is per_core[c, b % NPAIR, b // NPAIR]
    out = np.transpose(per_core, (0, 2, 1, 3, 4, 5))
    return np.ascontiguousarray(out.reshape(B, C, H, W))



# revision 2
# speedup vs baseline: 1.2971x; 1.2971x over previous
"""Trainium2 Bass kernel: binarized (XNOR/ReActNet-style) ResNet BasicBlock.

Computes, for x:[64,64,56,56] f32 and small per-channel parameters:

    out = PReLU_a(BN(conv3x3(sign(x + b0), scale * sign(w))) + x + b1) + b2

Distribution: data-parallel over the batch dim, 8 images per NeuronCore on
8 cores.  Per core, images (i, i+4) share the SBUF partition dim: channels
of the first image on partitions 0-63, channels of the second on 64-127.

I/O precision: x is cast to bf16 on the host (sign() and the residual both
survive bf16 exactly within the 2e-2 norm gate) and the output is stored
bf16 and upcast on the host, halving HBM traffic (the baseline bottleneck:
12.8MB ~ 39.6us measured; now 6.4MB ~ 20us).

Math folding (host side, all tiny tensors):
  - binarized weights sign(w) are pre-scaled by A_m = mean|w|_m * gamma_m /
    sqrt(var_m + eps)  (the BN multiplier), so PSUM holds BN-scaled conv.
    Products are +-A_m exactly, accumulated in fp32 PSUM -> only error is
    bf16 rounding of A_m itself (~2^-9 relative).
  - residual +x is accumulated into PSUM by an identity matmul streaming
    the bf16 x planes directly; the PE quadrant crossing aligns it with
    the conv halves.
  - sign(x) runs on the DVE as one fused bitwise op per half-image:
    act = (x & 0x8000) | 0x3f80  (+-1.0 in bf16); falls back to an ACT
    Sign op when bias0 is nonzero.
  - the whole epilogue u = prelu_a(psum + C2) is ONE ScalarE activation per
    slice reading PSUM directly (bias=C2 per-partition, alpha per-partition),
    writing bf16; C2 = beta - mean*inv + bias1.  bias2 (zero in practice)
    falls back to one extra DVE op when nonzero.

Engine budget per core/pass: PE streams 9 bf16 taps x 464 + 1 bf16 x 448
over 4 quadrant streams (~34us, the bottleneck), DMA moves 6.4MB (~20us),
ACT drains PSUM (~19us), DVE signs (~14us).  All DMAs ride the SP HWDGE
ring: 4 input loads up-front, per-pair output stores behind them.

On-chip layout: activations live in zero-padded 58x58 bf16 planes so each
3x3 tap is one contiguous 464-element matmul rhs slice; x and y live in
unpadded planes so HBM DMAs are 64 descriptors x 6.3KB contiguous.
Conv runs as 9 small matmuls per 8-row slice on 2x2 PE quadrants
(tile_position from partition bases); even/odd slices use complementary
quadrant pairs so four matmul streams run concurrently.
"""

import sys

if "/opt/trn_rl_repo" not in sys.path:
    sys.path.insert(0, "/opt/trn_rl_repo")

import numpy as np

import concourse.bass as bass
import concourse.bacc as bacc
import concourse.mybir as mybir
from concourse.tile import TileContext
from concourse.bass_utils import run_bass_kernel_spmd

AF = mybir.ActivationFunctionType
ALU = mybir.AluOpType
DT = mybir.dt

B, C, H, W = 64, 64, 56, 56
NCORES = 8
BPC = B // NCORES          # images per core
NPAIR = BPC // 2           # image pairs per core
HP, WP = H + 2, W + 2      # zero-padded plane 58x58
IMG = HP * WP              # 3364 elements per padded plane
PLN = H * W                # 3136 elements per unpadded plane
RB = 8                     # output rows per slice
NSL = H // RB              # 7 slices per image
NT = RB * WP               # 464: matmul free size (contiguous in padded space)
NI = RB * W                # 448: interior (valid) elements per slice
BN_EPS = 1e-5

_NC_CACHE = {}


def _build(flags, reps: int = 1, probe: str = ''):
    fast_prelu, fast_sign = flags
    nc = bacc.Bacc("TRN2", target_bir_lowering=False, debug=False)
    # pair-major layout: images (i, i+NPAIR) adjacent so one 128-partition
    # DMA covers a pair (host interleaves/deinterleaves)
    x_ext = nc.declare_dram_parameter("x", [NPAIR, 2, C, H, W], DT.bfloat16,
                                      isOutput=False)
    w_ext = nc.declare_dram_parameter("wts", [128, 9 * 64], DT.bfloat16, isOutput=False)
    i_ext = nc.declare_dram_parameter("wid", [128, 64], DT.bfloat16, isOutput=False)
    c_ext = nc.declare_dram_parameter("cst", [128, 8], DT.float32, isOutput=False)
    o_ext = nc.declare_dram_parameter("out", [NPAIR, 2, C, H, W], DT.bfloat16,
                                      isOutput=True)

    xg = x_ext.ap().rearrange("p b c h w -> p (b c) (h w)")
    og = o_ext.ap().rearrange("p b c h w -> p (b c) (h w)")

    with TileContext(nc) as tc:
        with tc.tile_pool(name="persist", bufs=1) as perst, \
             tc.tile_pool(name="psum", bufs=6, space="PSUM") as ppool:

            wts = perst.tile([128, 9 * 64], DT.bfloat16)
            nc.sync.dma_start(out=wts, in_=w_ext.ap())
            wid = perst.tile([128, 64], DT.bfloat16)
            nc.sync.dma_start(out=wid, in_=i_ext.ap())
            cst = perst.tile([128, 8], DT.float32)
            nc.sync.dma_start(out=cst, in_=c_ext.ap())
            c_ap = cst[:, 0:1]    # beta - mean*inv + bias1
            a_ap = cst[:, 1:2]    # PReLU alpha
            b2_ap = cst[:, 2:3]   # bias2 (nonzero only on the slow path)
            b0_ap = cst[:, 3:4]   # bias0 (nonzero only on the slow path)

            xf = perst.tile([128, NPAIR * PLN], DT.bfloat16)   # unpadded planes
            yb = perst.tile([128, NPAIR * PLN], DT.bfloat16)   # unpadded output
            act = perst.tile([128, NPAIR * IMG], DT.bfloat16)  # padded sign planes

            if probe == 'pe':
                # mark xf/act written so the PE-only probe's matmuls have
                # allocated (garbage) sources
                nc.vector.memset(xf[:, 0:1], 0.0)
                nc.vector.memset(act[:, 0:1], 0.0)
            if probe in ('dma', 'dma2'):
                # mark yb written: the DMA-only probe stores garbage yb so
                # loads (xf) and stores (yb) stay WAR-independent, matching
                # the real kernel's conveyor structure
                nc.vector.memset(yb[:, 0:1], 0.0)

            # zero the padding ring of every act plane
            for pr in range(NPAIR):
                v = act[:, pr * IMG:(pr + 1) * IMG].rearrange(
                    "p (h w) -> p h w", w=WP)
                nc.vector.memset(v[:, 0:1, :], 0.0)
                nc.vector.memset(v[:, HP - 1:HP, :], 0.0)
                nc.vector.memset(v[:, 1:HP - 1, 0:1], 0.0)
                nc.vector.memset(v[:, 1:HP - 1, WP - 1:WP], 0.0)

            def emit_load(pr):
                ub = pr * PLN
                nc.sync.dma_start(out=xf[:, ub:ub + PLN], in_=xg[pr])

            def emit_sign(pr, half):
                # sign of a half-image (28 rows) in one instruction
                ub = pr * PLN
                ab = pr * IMG
                h0 = half * (H // 2)
                nr = H // 2
                src = xf[:, ub + h0 * W:ub + (h0 + nr) * W].rearrange(
                    "p (r c) -> p r c", c=W)
                adst = act[:, ab:ab + IMG].rearrange(
                    "p (h w) -> p h w", w=WP)[:, 1 + h0:1 + h0 + nr, 1:1 + W]
                if fast_sign:
                    # bf16 sign via bit ops on the DVE: keep the sign bit,
                    # force the exponent/mantissa of 1.0
                    nc.vector.tensor_scalar(
                        adst.bitcast(DT.uint16), src.bitcast(DT.uint16),
                        0x8000, 0x3f80, ALU.bitwise_and, ALU.bitwise_or)
                else:
                    nc.scalar.activation(out=adst, in_=src, func=AF.Sign,
                                         bias=b0_ap, scale=1.0)

            def mm_args(s, t):
                pr, rc = divmod(s, NSL)
                h0 = rc * RB
                if t == 9:      # identity (residual) step: bf16 from xf
                    u0 = pr * PLN + h0 * W
                    la = wid[0:64]
                    lb = wid[64:128]
                    ra = xf[0:64, u0:u0 + NI]
                    rb = xf[64:128, u0:u0 + NI]
                    return la, lb, ra, rb, None, None, s % 2
                dh, dw = t // 3 - 1, t % 3 - 1
                off = pr * IMG + (h0 + dh + 1) * WP + dw
                s0 = 1 if (rc == 0 and dh == -1 and dw == -1) else 0
                s1 = NT - 1 if (rc == NSL - 1 and dh == 1 and dw == 1) else NT
                la = wts[0:64, t * 64:(t + 1) * 64]
                lb = wts[64:128, t * 64:(t + 1) * 64]
                ra = act[0:64, off + s0:off + s1]
                rb = act[64:128, off + s0:off + s1]
                return la, lb, ra, rb, s0, s1, s % 2

            def emit_mms(slice_group):
                # interleave matmuls of an even+odd slice pair so all four
                # PE quadrants stream concurrently (starts are pc-monotone;
                # disjoint tile_positions overlap)
                pss = {}
                for s in slice_group:
                    pss[s] = ppool.tile([128, NT], DT.float32, tag="ps",
                                        name=f"ps{s}")
                # center tap first: never range-trimmed, so start=True clears
                # the whole bank before the trimmed corner taps accumulate
                for t in (4, 0, 1, 2, 3, 5, 6, 7, 8, 9):
                    for s in slice_group:
                        la, lb, ra, rb, s0, s1, odd = mm_args(s, t)
                        ps = pss[s]
                        if t == 9:
                            # write the 448-elem interior (rows of 56 within
                            # the 58-wide padded window)
                            pv = ps.rearrange("p (r c) -> p r c", c=WP)[:, :, 1:1 + W]
                            pa = pv[64:128] if odd else pv[0:64]
                            pb = pv[0:64] if odd else pv[64:128]
                        else:
                            pa = ps[64:128, s0:s1] if odd else ps[0:64, s0:s1]
                            pb = ps[0:64, s0:s1] if odd else ps[64:128, s0:s1]
                        nc.tensor.matmul(pa, la, ra, start=(t == 4),
                                         stop=(t == 9), skip_group_check=True)
                        nc.tensor.matmul(pb, lb, rb, start=(t == 4),
                                         stop=(t == 9), skip_group_check=True)
                return pss

            def emit_epilogue(s, ps):
                pr, rc = divmod(s, NSL)
                h0 = rc * RB
                u0 = pr * PLN + h0 * W
                ps_i = ps.rearrange("p (r c) -> p r c", c=WP)[:, :, 1:1 + W]
                yv = yb[:, u0:u0 + NI].rearrange("p (r c) -> p r c", c=W)
                # y = prelu_a(psum + C2): one ACT op drains PSUM, adds the
                # folded BN/bias constant and applies per-channel PReLU
                nc.scalar.activation(out=yv, in_=ps_i, func=AF.Prelu,
                                     bias=c_ap, scale=1.0, alpha=a_ap)
                if not fast_prelu:
                    # + bias2 (only when nonzero)
                    yf = yb[:, u0:u0 + NI]
                    nc.vector.tensor_scalar_add(yf, yf, b2_ap)

            def emit_store(pr, eng=None):
                # output store, split by slice parity: odd-parity slices have
                # swapped halves (image B on partitions 0-63) from the crossed
                # PE quadrants.  Normal-parity chunks go out as one merged
                # 128-partition DMA; swapped-parity chunks as two crossed
                # 64-partition DMAs.
                eng = eng if eng is not None else nc.sync
                ub = pr * PLN
                ov = og[pr].rearrange("q (k n) -> q k n", n=NI)
                yv = yb[:, ub:ub + PLN].rearrange("q (k n) -> q k n", n=NI)
                pn = pr % 2          # rc parity whose layout is normal [A|B]
                psw = 1 - pn
                eng.dma_start(out=ov[:, pn:NSL:2], in_=yv[:, pn:NSL:2])
                eng.dma_start(out=ov[0:64, psw:NSL:2], in_=yv[64:128, psw:NSL:2])
                eng.dma_start(out=ov[64:128, psw:NSL:2], in_=yv[0:64, psw:NSL:2])

            def emit_compute_all():
                if probe == 'pe':
                    for s0 in range(0, NPAIR * NSL - 1, 2):
                        emit_mms([s0, s0 + 1])
                    return
                if probe == 'dma':
                    for pr in range(NPAIR):
                        emit_store(pr)
                    return
                if probe == 'dma2':
                    # conveyor with stores on the second HWDGE ring (ACT)
                    for pr in range(NPAIR):
                        emit_store(pr, eng=nc.scalar)
                    return
                nsl_tot = NPAIR * NSL
                s = 0
                while s < nsl_tot:
                    group = [s] if s + 1 >= nsl_tot else [s, s + 1]
                    if probe == 'nope':
                        pss = {g: ppool.tile([128, NT], DT.float32, tag="ps",
                                             name=f"ps{g}") for g in group}
                        for g in group:
                            nc.vector.memset(pss[g][:, 0:1], 0.0)
                    else:
                        pss = emit_mms(group)
                    for g in group:
                        emit_epilogue(g, pss[g])
                    # prefetch next pair's signs (queued behind this group's
                    # drains so they never head-of-line block them on a
                    # pending input load): at slices 0 and 2 of pair pr, emit
                    # the two half-image signs of pair pr+1
                    for g in group:
                        pr_g, rc_g = divmod(g, NSL)
                        if rc_g in (0, 2) and pr_g + 1 < NPAIR:
                            emit_sign(pr_g + 1, rc_g // 2)
                    for g in group:
                        if (g + 1) % NSL == 0:
                            emit_store(g // NSL)
                    s += len(group)

            for _ in range(reps):
                if probe != 'pe':
                    # loads all up-front on the SP ring; pair-0 signs up-front
                    # (later pairs' signs are interleaved into the slice loop)
                    for pr in range(NPAIR):
                        emit_load(pr)
                    if probe not in ('dma', 'dma2'):
                        emit_sign(0, 0)
                        emit_sign(0, 1)
                emit_compute_all()

    nc.compile()
    return nc


def _get_nc(flags, reps: int = 1, probe: str = ''):
    key = (flags, reps, probe)
    if key not in _NC_CACHE:
        _NC_CACHE[key] = _build(flags, reps, probe)
    return _NC_CACHE[key]


def _prepare(x, bias0, w, gamma, beta, run_mean, run_var, bias1, alpha, bias2):
    bf16 = DT.np(DT.bfloat16)
    x = np.asarray(x, np.float32)
    w = np.asarray(w, np.float32)
    sw = np.sign(w)                                   # [P, C, 3, 3]
    scale = np.abs(w).mean(axis=(1, 2, 3))            # [P]
    inv = np.asarray(gamma, np.float32) / np.sqrt(
        np.asarray(run_var, np.float32) + np.float32(BN_EPS))
    A = (scale * inv).astype(np.float32)
    b1 = np.asarray(bias1, np.float32).reshape(-1)
    b2 = np.asarray(bias2, np.float32).reshape(-1)
    al = np.asarray(alpha, np.float32).reshape(-1)
    b0 = np.asarray(bias0, np.float32).reshape(-1)
    Cc = (np.asarray(beta, np.float32) -
          np.asarray(run_mean, np.float32) * inv + b1).astype(np.float32)

    wt = np.zeros((128, 9 * 64), np.float32)
    for t in range(9):
        blk = (sw[:, :, t // 3, t % 3] * A[:, None]).T      # [C, P]
        wt[0:64, t * 64:(t + 1) * 64] = blk
        wt[64:128, t * 64:(t + 1) * 64] = blk
    wt_bf = np.ascontiguousarray(wt.astype(bf16))
    wid = np.zeros((128, 64), np.float32)
    wid[0:64] = np.eye(64, dtype=np.float32)
    wid[64:128] = np.eye(64, dtype=np.float32)
    wid = np.ascontiguousarray(wid.astype(bf16))

    cst = np.zeros((128, 8), np.float32)
    for half in range(2):
        sl = slice(half * 64, half * 64 + 64)
        cst[sl, 0] = Cc
        cst[sl, 1] = al
        cst[sl, 2] = b2
        cst[sl, 3] = b0

    fast_prelu = bool(np.all(b2 == 0.0))
    fast_sign = bool(np.all(b0 == 0.0))
    xb = x.astype(bf16)
    in_maps = []
    for c in range(NCORES):
        xc = xb[c * BPC:(c + 1) * BPC]
        # pair-major: [NPAIR, 2, C, H, W] with images (p, p+NPAIR) adjacent
        xpm = np.ascontiguousarray(
            np.stack([xc[0:NPAIR], xc[NPAIR:BPC]], axis=1))
        in_maps.append({"x": xpm, "wts": wt_bf, "wid": wid, "cst": cst})
    return in_maps, (fast_prelu, fast_sign)


_RUNNER_CACHE = {}


def _make_runner(nc, n_cores=NCORES):
    """Build a reusable jitted executor for `nc` (one XLA trace, NEFF cached)."""
    import jax
    from jax.sharding import Mesh, PartitionSpec, NamedSharding
    from jax.experimental.shard_map import shard_map
    from concourse import bass2jax

    bass2jax.install_neuronx_cc_hook()
    partition_name = nc.partition_id_tensor.name if nc.partition_id_tensor else None
    in_names, out_names, out_avals, zero_outs = [], [], [], []
    for alloc in nc.m.functions[0].allocations:
        if not isinstance(alloc, mybir.MemoryLocationSet):
            continue
        name = alloc.memorylocations[0].name
        if alloc.kind == "ExternalInput":
            if name != partition_name:
                in_names.append(name)
        elif alloc.kind == "ExternalOutput":
            out_names.append(name)
            shape = tuple(alloc.tensor_shape)
            dtype = mybir.dt.np(alloc.dtype)
            out_avals.append(jax.core.ShapedArray(shape, dtype))
            zero_outs.append(np.zeros(shape, dtype))
    n_params = len(in_names)
    all_in = list(in_names) + out_names + ([partition_name] if partition_name else [])

    def _body(*args):
        operands = list(args)
        if partition_name is not None:
            operands.append(bass2jax.partition_id_tensor())
        outs = bass2jax._bass_exec_p.bind(
            *operands,
            out_avals=tuple(out_avals),
            in_names=tuple(all_in),
            out_names=tuple(out_names),
            lowering_input_output_aliases=(),
            sim_require_finite=True,
            sim_require_nnan=True,
            nc=nc,
        )
        return tuple(outs)

    devices = jax.devices()[:n_cores]
    mesh = Mesh(np.asarray(devices), ("core",))
    nin = n_params + len(out_names)
    f = jax.jit(shard_map(
        _body, mesh=mesh,
        in_specs=(PartitionSpec("core"),) * nin,
        out_specs=(PartitionSpec("core"),) * len(out_names),
        check_rep=False))
    sh = NamedSharding(mesh, PartitionSpec("core"))
    concat_zeros = [
        jax.device_put(np.zeros((n_cores * z.shape[0], *z.shape[1:]), z.dtype), sh)
        for z in zero_outs
    ]

    def run(in_maps):
        concat_in = [
            np.concatenate([np.asarray(in_maps[c][nm]) for c in range(n_cores)],
                           axis=0)
            for nm in in_names
        ]
        args = [jax.device_put(a, sh) for a in concat_in] + concat_zeros
        outs = f(*args)
        jax.block_until_ready(outs)
        oi = out_names.index("out")
        full = np.asarray(outs[oi])
        return full.reshape(n_cores, *out_avals[oi].shape)

    run.jit_fn = f
    run.sharding = sh
    run.in_names = in_names
    run.out_names = out_names
    run.zero_args = concat_zeros
    return run


def _get_runner(flags, reps: int = 1, probe: str = ''):
    key = (flags, reps, probe)
    if key not in _RUNNER_CACHE:
        _RUNNER_CACHE[key] = _make_runner(_get_nc(flags, reps, probe))
    return _RUNNER_CACHE[key]


def _run(inputs: dict, trace: bool = False, reps: int = 1, **spmd_kwargs):
    """Legacy path through run_bass_kernel_spmd (used for debugging)."""
    in_maps, flags = _prepare(**inputs)
    nc = _get_nc(flags, reps)
    res = run_bass_kernel_spmd(nc, in_maps, list(range(NCORES)),
                               trace=trace, **spmd_kwargs)
    per_core = np.stack([res.results[c]["out"] for c in range(NCORES)], axis=0)
    out = np.transpose(per_core, (0, 2, 1, 3, 4, 5)).reshape(B, C, H, W)
    return out.astype(np.float32), res


def kernel(**inputs) -> np.ndarray:
    in_maps, flags = _prepare(**inputs)
    runner = _get_runner(flags)
    per_core = runner(in_maps)      # [NCORES, NPAIR, 2, C, H, W] bf16
    # undo pair-major: image b of core c is per_core[c, b % NPAIR, b // NPAIR]
    out = np.transpose(per_core, (0, 2, 1, 3, 4, 5))
    return np.ascontiguousarray(
        out.reshape(B, C, H, W).astype(np.float32))


# revision 3
# speedup vs baseline: 1.3940x; 1.0747x over previous
"""Trainium2 Bass kernel: binarized (XNOR/ReActNet-style) ResNet BasicBlock.

Computes, for x:[64,64,56,56] f32 and small per-channel parameters:

    out = PReLU_a(BN(conv3x3(sign(x + b0), scale * sign(w))) + x + b1) + b2

Distribution: data-parallel over the batch dim, 8 images per NeuronCore on
8 cores.  Per core, images (i, i+4) share the SBUF partition dim: channels
of the first image on partitions 0-63, channels of the second on 64-127.

I/O precision: x is cast to bf16 on the host (sign() and the residual both
survive bf16 exactly within the 2e-2 norm gate) and the output is stored
bf16 and upcast on the host, halving HBM traffic (the baseline bottleneck:
12.8MB ~ 39.6us measured; now 6.4MB ~ 20us).

Math folding (host side, all tiny tensors):
  - binarized weights sign(w) are pre-scaled by A_m = mean|w|_m * gamma_m /
    sqrt(var_m + eps)  (the BN multiplier), so PSUM holds BN-scaled conv.
    Products are +-A_m exactly, accumulated in fp32 PSUM -> only error is
    bf16 rounding of A_m itself (~2^-9 relative).
  - residual +x is accumulated into PSUM by an identity matmul streaming
    the bf16 x planes directly; the PE quadrant crossing aligns it with
    the conv halves.
  - sign(x) runs on the DVE as one fused bitwise op per half-image:
    act = (x & 0x8000) | 0x3f80  (+-1.0 in bf16); falls back to an ACT
    Sign op when bias0 is nonzero.
  - the whole epilogue u = prelu_a(psum + C2) is ONE ScalarE activation per
    slice reading PSUM directly (bias=C2 per-partition, alpha per-partition),
    writing bf16; C2 = beta - mean*inv + bias1.  bias2 (zero in practice)
    falls back to one extra DVE op when nonzero.

Engine budget per core/pass: PE streams 9 bf16 taps x 464 + 1 bf16 x 448
over 4 quadrant streams (~34us, the bottleneck), DMA moves 6.4MB (~20us),
ACT drains PSUM (~19us), DVE signs (~14us).  All DMAs ride the SP HWDGE
ring: 4 input loads up-front, per-pair output stores behind them.

On-chip layout: activations live in zero-padded 58x58 bf16 planes so each
3x3 tap is one contiguous 464-element matmul rhs slice; x and y live in
unpadded planes so HBM DMAs are 64 descriptors x 6.3KB contiguous.
Conv runs as 9 small matmuls per 8-row slice on 2x2 PE quadrants
(tile_position from partition bases); even/odd slices use complementary
quadrant pairs so four matmul streams run concurrently.
"""

import sys

if "/opt/trn_rl_repo" not in sys.path:
    sys.path.insert(0, "/opt/trn_rl_repo")

import numpy as np

import concourse.bass as bass
import concourse.bacc as bacc
import concourse.mybir as mybir
from concourse.tile import TileContext
from concourse.bass_utils import run_bass_kernel_spmd

AF = mybir.ActivationFunctionType
ALU = mybir.AluOpType
DT = mybir.dt

B, C, H, W = 64, 64, 56, 56
NCORES = 8
BPC = B // NCORES          # images per core
NPAIR = BPC // 2           # image pairs per core
HP, WP = H + 2, W + 2      # zero-padded plane 58x58
IMG = HP * WP              # 3364 elements per padded plane
PLN = H * W                # 3136 elements per unpadded plane
RB = 8                     # output rows per slice
NSL = H // RB              # 7 slices per image
NT = RB * WP               # 464: matmul free size (contiguous in padded space)
NI = RB * W                # 448: interior (valid) elements per slice
BN_EPS = 1e-5

_NC_CACHE = {}


def _build(flags, reps: int = 1, probe: str = ''):
    fast_prelu, fast_sign = flags
    nc = bacc.Bacc("TRN2", target_bir_lowering=False, debug=False)
    # pair-major layout: images (i, i+NPAIR) adjacent so one 128-partition
    # DMA covers a pair (host interleaves/deinterleaves)
    x_ext = nc.declare_dram_parameter("x", [NPAIR, 2, C, H, W], DT.bfloat16,
                                      isOutput=False)
    w_ext = nc.declare_dram_parameter("wts", [128, 9 * 64], DT.bfloat16, isOutput=False)
    i_ext = nc.declare_dram_parameter("wid", [128, 64], DT.bfloat16, isOutput=False)
    c_ext = nc.declare_dram_parameter("cst", [128, 8], DT.float32, isOutput=False)
    o_ext = nc.declare_dram_parameter("out", [NPAIR, 2, C, H, W], DT.bfloat16,
                                      isOutput=True)

    xg = x_ext.ap().rearrange("p b c h w -> p (b c) (h w)")
    og = o_ext.ap().rearrange("p b c h w -> p (b c) (h w)")

    with TileContext(nc) as tc:
        with tc.tile_pool(name="persist", bufs=1) as perst, \
             tc.tile_pool(name="psum", bufs=8, space="PSUM") as ppool:

            wts = perst.tile([128, 9 * 64], DT.bfloat16)
            nc.sync.dma_start(out=wts, in_=w_ext.ap())
            wid = perst.tile([128, 64], DT.bfloat16)
            nc.sync.dma_start(out=wid, in_=i_ext.ap())
            cst = perst.tile([128, 8], DT.float32)
            nc.sync.dma_start(out=cst, in_=c_ext.ap())
            c_ap = cst[:, 0:1]    # beta - mean*inv + bias1
            a_ap = cst[:, 1:2]    # PReLU alpha
            b2_ap = cst[:, 2:3]   # bias2 (nonzero only on the slow path)
            b0_ap = cst[:, 3:4]   # bias0 (nonzero only on the slow path)

            xf = perst.tile([128, NPAIR * PLN], DT.bfloat16)   # unpadded planes
            yb = perst.tile([128, NPAIR * PLN], DT.bfloat16)   # unpadded output
            act = perst.tile([128, NPAIR * IMG], DT.bfloat16)  # padded sign planes

            if probe == 'pe':
                # mark xf/act written so the PE-only probe's matmuls have
                # allocated (garbage) sources
                nc.vector.memset(xf[:, 0:1], 0.0)
                nc.vector.memset(act[:, 0:1], 0.0)
            if probe in ('dma', 'dma2'):
                # mark yb written: the DMA-only probe stores garbage yb so
                # loads (xf) and stores (yb) stay WAR-independent, matching
                # the real kernel's conveyor structure
                nc.vector.memset(yb[:, 0:1], 0.0)

            # zero the padding ring of every act plane
            for pr in range(NPAIR):
                v = act[:, pr * IMG:(pr + 1) * IMG].rearrange(
                    "p (h w) -> p h w", w=WP)
                nc.vector.memset(v[:, 0:1, :], 0.0)
                nc.vector.memset(v[:, HP - 1:HP, :], 0.0)
                nc.vector.memset(v[:, 1:HP - 1, 0:1], 0.0)
                nc.vector.memset(v[:, 1:HP - 1, WP - 1:WP], 0.0)

            def emit_load(pr):
                ub = pr * PLN
                nc.sync.dma_start(out=xf[:, ub:ub + PLN], in_=xg[pr])

            def emit_sign(pr, half):
                # sign of a half-image (28 rows) in one instruction
                ub = pr * PLN
                ab = pr * IMG
                h0 = half * (H // 2)
                nr = H // 2
                src = xf[:, ub + h0 * W:ub + (h0 + nr) * W].rearrange(
                    "p (r c) -> p r c", c=W)
                adst = act[:, ab:ab + IMG].rearrange(
                    "p (h w) -> p h w", w=WP)[:, 1 + h0:1 + h0 + nr, 1:1 + W]
                if fast_sign:
                    # bf16 sign via bit ops on the DVE: keep the sign bit,
                    # force the exponent/mantissa of 1.0
                    nc.vector.tensor_scalar(
                        adst.bitcast(DT.uint16), src.bitcast(DT.uint16),
                        0x8000, 0x3f80, ALU.bitwise_and, ALU.bitwise_or)
                else:
                    nc.scalar.activation(out=adst, in_=src, func=AF.Sign,
                                         bias=b0_ap, scale=1.0)

            def mm_args(s, t):
                pr, rc = divmod(s, NSL)
                h0 = rc * RB
                if t == 9:      # identity (residual) step: bf16 from xf
                    u0 = pr * PLN + h0 * W
                    la = wid[0:64]
                    lb = wid[64:128]
                    ra = xf[0:64, u0:u0 + NI]
                    rb = xf[64:128, u0:u0 + NI]
                    return la, lb, ra, rb, None, None, s % 2
                dh, dw = t // 3 - 1, t % 3 - 1
                off = pr * IMG + (h0 + dh + 1) * WP + dw
                s0 = 1 if (rc == 0 and dh == -1 and dw == -1) else 0
                s1 = NT - 1 if (rc == NSL - 1 and dh == 1 and dw == 1) else NT
                la = wts[0:64, t * 64:(t + 1) * 64]
                lb = wts[64:128, t * 64:(t + 1) * 64]
                ra = act[0:64, off + s0:off + s1]
                rb = act[64:128, off + s0:off + s1]
                return la, lb, ra, rb, s0, s1, s % 2

            def emit_mms(slice_group):
                # interleave matmuls of an even+odd slice pair so all four
                # PE quadrants stream concurrently (starts are pc-monotone;
                # disjoint tile_positions overlap)
                pss = {}
                for s in slice_group:
                    pss[s] = ppool.tile([128, NT], DT.float32, tag="ps",
                                        name=f"ps{s}")
                # center tap first: never range-trimmed, so start=True clears
                # the whole bank before the trimmed corner taps accumulate
                for t in (4, 0, 1, 2, 3, 5, 6, 7, 8, 9):
                    for s in slice_group:
                        la, lb, ra, rb, s0, s1, odd = mm_args(s, t)
                        ps = pss[s]
                        if t == 9:
                            # write the 448-elem interior (rows of 56 within
                            # the 58-wide padded window)
                            pv = ps.rearrange("p (r c) -> p r c", c=WP)[:, :, 1:1 + W]
                            pa = pv[64:128] if odd else pv[0:64]
                            pb = pv[0:64] if odd else pv[64:128]
                        else:
                            pa = ps[64:128, s0:s1] if odd else ps[0:64, s0:s1]
                            pb = ps[0:64, s0:s1] if odd else ps[64:128, s0:s1]
                        nc.tensor.matmul(pa, la, ra, start=(t == 4),
                                         stop=(t == 9), skip_group_check=True)
                        nc.tensor.matmul(pb, lb, rb, start=(t == 4),
                                         stop=(t == 9), skip_group_check=True)
                return pss

            def emit_epilogue(s, ps):
                pr, rc = divmod(s, NSL)
                h0 = rc * RB
                u0 = pr * PLN + h0 * W
                ps_i = ps.rearrange("p (r c) -> p r c", c=WP)[:, :, 1:1 + W]
                yv = yb[:, u0:u0 + NI].rearrange("p (r c) -> p r c", c=W)
                # y = prelu_a(psum + C2): one ACT op drains PSUM, adds the
                # folded BN/bias constant and applies per-channel PReLU
                nc.scalar.activation(out=yv, in_=ps_i, func=AF.Prelu,
                                     bias=c_ap, scale=1.0, alpha=a_ap)
                if not fast_prelu:
                    # + bias2 (only when nonzero)
                    yf = yb[:, u0:u0 + NI]
                    nc.vector.tensor_scalar_add(yf, yf, b2_ap)

            def emit_store(pr, eng=None):
                # output store, split by slice parity: odd-parity slices have
                # swapped halves (image B on partitions 0-63) from the crossed
                # PE quadrants.  Normal-parity chunks go out as one merged
                # 128-partition DMA; swapped-parity chunks as two crossed
                # 64-partition DMAs.
                eng = eng if eng is not None else nc.sync
                ub = pr * PLN
                ov = og[pr].rearrange("q (k n) -> q k n", n=NI)
                yv = yb[:, ub:ub + PLN].rearrange("q (k n) -> q k n", n=NI)
                pn = pr % 2          # rc parity whose layout is normal [A|B]
                psw = 1 - pn
                eng.dma_start(out=ov[:, pn:NSL:2], in_=yv[:, pn:NSL:2])
                eng.dma_start(out=ov[0:64, psw:NSL:2], in_=yv[64:128, psw:NSL:2])
                eng.dma_start(out=ov[64:128, psw:NSL:2], in_=yv[0:64, psw:NSL:2])

            def emit_compute_all():
                if probe == 'pe':
                    for s0 in range(0, NPAIR * NSL - 1, 2):
                        emit_mms([s0, s0 + 1])
                    return
                if probe == 'dma':
                    for pr in range(NPAIR):
                        emit_store(pr)
                    return
                if probe == 'dma2':
                    # conveyor with stores on the second HWDGE ring (ACT)
                    for pr in range(NPAIR):
                        emit_store(pr, eng=nc.scalar)
                    return
                nsl_tot = NPAIR * NSL
                s = 0
                while s < nsl_tot:
                    group = [s] if s + 1 >= nsl_tot else [s, s + 1]
                    if probe == 'nope':
                        pss = {g: ppool.tile([128, NT], DT.float32, tag="ps",
                                             name=f"ps{g}") for g in group}
                        for g in group:
                            nc.vector.memset(pss[g][:, 0:1], 0.0)
                    else:
                        pss = emit_mms(group)
                    for g in group:
                        emit_epilogue(g, pss[g])
                    # prefetch next pair's signs (queued behind this group's
                    # drains so they never head-of-line block them on a
                    # pending input load): at slices 0 and 2 of pair pr, emit
                    # the two half-image signs of pair pr+1
                    for g in group:
                        pr_g, rc_g = divmod(g, NSL)
                        if rc_g in (0, 2) and pr_g + 1 < NPAIR:
                            emit_sign(pr_g + 1, rc_g // 2)
                    for g in group:
                        if (g + 1) % NSL == 0:
                            emit_store(g // NSL)
                    s += len(group)

            for _ in range(reps):
                if probe != 'pe':
                    # loads all up-front on the SP ring; pair-0 signs up-front
                    # (later pairs' signs are interleaved into the slice loop)
                    for pr in range(NPAIR):
                        emit_load(pr)
                    if probe not in ('dma', 'dma2'):
                        emit_sign(0, 0)
                        emit_sign(0, 1)
                emit_compute_all()

    nc.compile()
    return nc


def _get_nc(flags, reps: int = 1, probe: str = ''):
    key = (flags, reps, probe)
    if key not in _NC_CACHE:
        _NC_CACHE[key] = _build(flags, reps, probe)
    return _NC_CACHE[key]


def _prepare(x, bias0, w, gamma, beta, run_mean, run_var, bias1, alpha, bias2):
    bf16 = DT.np(DT.bfloat16)
    x = np.asarray(x, np.float32)
    w = np.asarray(w, np.float32)
    sw = np.sign(w)                                   # [P, C, 3, 3]
    scale = np.abs(w).mean(axis=(1, 2, 3))            # [P]
    inv = np.asarray(gamma, np.float32) / np.sqrt(
        np.asarray(run_var, np.float32) + np.float32(BN_EPS))
    A = (scale * inv).astype(np.float32)
    b1 = np.asarray(bias1, np.float32).reshape(-1)
    b2 = np.asarray(bias2, np.float32).reshape(-1)
    al = np.asarray(alpha, np.float32).reshape(-1)
    b0 = np.asarray(bias0, np.float32).reshape(-1)
    Cc = (np.asarray(beta, np.float32) -
          np.asarray(run_mean, np.float32) * inv + b1).astype(np.float32)

    wt = np.zeros((128, 9 * 64), np.float32)
    for t in range(9):
        blk = (sw[:, :, t // 3, t % 3] * A[:, None]).T      # [C, P]
        wt[0:64, t * 64:(t + 1) * 64] = blk
        wt[64:128, t * 64:(t + 1) * 64] = blk
    wt_bf = np.ascontiguousarray(wt.astype(bf16))
    wid = np.zeros((128, 64), np.float32)
    wid[0:64] = np.eye(64, dtype=np.float32)
    wid[64:128] = np.eye(64, dtype=np.float32)
    wid = np.ascontiguousarray(wid.astype(bf16))

    cst = np.zeros((128, 8), np.float32)
    for half in range(2):
        sl = slice(half * 64, half * 64 + 64)
        cst[sl, 0] = Cc
        cst[sl, 1] = al
        cst[sl, 2] = b2
        cst[sl, 3] = b0

    fast_prelu = bool(np.all(b2 == 0.0))
    fast_sign = bool(np.all(b0 == 0.0))
    xb = x.astype(bf16)
    in_maps = []
    for c in range(NCORES):
        xc = xb[c * BPC:(c + 1) * BPC]
        # pair-major: [NPAIR, 2, C, H, W] with images (p, p+NPAIR) adjacent
        xpm = np.ascontiguousarray(
            np.stack([xc[0:NPAIR], xc[NPAIR:BPC]], axis=1))
        in_maps.append({"x": xpm, "wts": wt_bf, "wid": wid, "cst": cst})
    return in_maps, (fast_prelu, fast_sign)


_RUNNER_CACHE = {}


def _make_runner(nc, n_cores=NCORES):
    """Build a reusable jitted executor for `nc` (one XLA trace, NEFF cached)."""
    import jax
    from jax.sharding import Mesh, PartitionSpec, NamedSharding
    from jax.experimental.shard_map import shard_map
    from concourse import bass2jax

    bass2jax.install_neuronx_cc_hook()
    partition_name = nc.partition_id_tensor.name if nc.partition_id_tensor else None
    in_names, out_names, out_avals, zero_outs = [], [], [], []
    for alloc in nc.m.functions[0].allocations:
        if not isinstance(alloc, mybir.MemoryLocationSet):
            continue
        name = alloc.memorylocations[0].name
        if alloc.kind == "ExternalInput":
            if name != partition_name:
                in_names.append(name)
        elif alloc.kind == "ExternalOutput":
            out_names.append(name)
            shape = tuple(alloc.tensor_shape)
            dtype = mybir.dt.np(alloc.dtype)
            out_avals.append(jax.core.ShapedArray(shape, dtype))
            zero_outs.append(np.zeros(shape, dtype))
    n_params = len(in_names)
    all_in = list(in_names) + out_names + ([partition_name] if partition_name else [])

    def _body(*args):
        operands = list(args)
        if partition_name is not None:
            operands.append(bass2jax.partition_id_tensor())
        outs = bass2jax._bass_exec_p.bind(
            *operands,
            out_avals=tuple(out_avals),
            in_names=tuple(all_in),
            out_names=tuple(out_names),
            lowering_input_output_aliases=(),
            sim_require_finite=True,
            sim_require_nnan=True,
            nc=nc,
        )
        return tuple(outs)

    devices = jax.devices()[:n_cores]
    mesh = Mesh(np.asarray(devices), ("core",))
    nin = n_params + len(out_names)
    f = jax.jit(shard_map(
        _body, mesh=mesh,
        in_specs=(PartitionSpec("core"),) * nin,
        out_specs=(PartitionSpec("core"),) * len(out_names),
        check_rep=False))
    sh = NamedSharding(mesh, PartitionSpec("core"))
    concat_zeros = [
        jax.device_put(np.zeros((n_cores * z.shape[0], *z.shape[1:]), z.dtype), sh)
        for z in zero_outs
    ]

    def run(in_maps):
        concat_in = [
            np.concatenate([np.asarray(in_maps[c][nm]) for c in range(n_cores)],
                           axis=0)
            for nm in in_names
        ]
        args = [jax.device_put(a, sh) for a in concat_in] + concat_zeros
        outs = f(*args)
        jax.block_until_ready(outs)
        oi = out_names.index("out")
        full = np.asarray(outs[oi])
        return full.reshape(n_cores, *out_avals[oi].shape)

    run.jit_fn = f
    run.sharding = sh
    run.in_names = in_names
    run.out_names = out_names
    run.zero_args = concat_zeros
    return run


def _get_runner(flags, reps: int = 1, probe: str = ''):
    key = (flags, reps, probe)
    if key not in _RUNNER_CACHE:
        _RUNNER_CACHE[key] = _make_runner(_get_nc(flags, reps, probe))
    return _RUNNER_CACHE[key]


def _run(inputs: dict, trace: bool = False, reps: int = 1, **spmd_kwargs):
    """Legacy path through run_bass_kernel_spmd (used for debugging)."""
    in_maps, flags = _prepare(**inputs)
    nc = _get_nc(flags, reps)
    res = run_bass_kernel_spmd(nc, in_maps, list(range(NCORES)),
                               trace=trace, **spmd_kwargs)
    per_core = np.stack([res.results[c]["out"] for c in range(NCORES)], axis=0)
    out = np.transpose(per_core, (0, 2, 1, 3, 4, 5)).reshape(B, C, H, W)
    return out.astype(np.float32), res


def kernel(**inputs) -> np.ndarray:
    in_maps, flags = _prepare(**inputs)
    runner = _get_runner(flags)
    per_core = runner(in_maps)      # [NCORES, NPAIR, 2, C, H, W] bf16
    # undo pair-major: image b of core c is per_core[c, b % NPAIR, b // NPAIR]
    out = np.transpose(per_core, (0, 2, 1, 3, 4, 5))
    return np.ascontiguousarray(
        out.reshape(B, C, H, W).astype(np.float32))
